# revision 1
# baseline (speedup 1.0000x reference)
"""Trainium2 Bass kernel for nn_DeepUDI (RGAT+GRU message passing), 8-core SPMD.

Sharding: nodes (dim 0) split across 8 cores, 256 nodes = 512 (node,relation)
pairs per core. The full node-state table h=embed[x] is tiny (512KB) so the
neighbor gather is done on host while sharding (graph-parallel, no collectives).

Algebraic restructuring (host-side, exact in fp32):
  attention scores_k = hn_k . (A @ h)  with  A = w @ kw @ qw^T @ w^T  [64,64]
  folds qw,kw (16K elems/pair) into A (4K elems/pair): ~25% less HBM traffic.
  df = (w^T @ (hn^T @ E)) / sum(E);  E = exp(scores)  (|scores|<~30, no max sub)
  GRU gates on DVE with pairs-on-partitions; per-pair matvecs on TensorE with
  per-pair stationary (LDW) + 1-col moving, outputs batched into PSUM columns.
"""

import numpy as np

N, R, K, D, F, D2 = 2048, 2, 32, 64, 64, 128
P_ALL = N * R           # 4096 pairs
NCORES = 8
PPC = P_ALL // NCORES   # 512 pairs/core
NPC = N // NCORES       # 256 nodes/core
TILE = 128              # pairs per DVE tile
GRP = 32                # pairs per TensorE stationary group
NT = PPC // TILE        # 4 tiles/core
NG = PPC // GRP         # 16 groups/core
GPT = TILE // GRP       # 4 groups per tile

_cache = {}


def _build(reps=1):
    import contextlib
    import concourse.mybir as mybir
    import concourse.tile as tile
    from concourse import bacc
    from concourse.masks import make_identity

    fp32 = mybir.dt.float32
    nc = bacc.Bacc(
        "TRN2", target_bir_lowering=False, debug=False, num_devices=NCORES
    )

    # ---- DRAM I/O (per-core shards) ----
    d_wS = nc.dram_tensor("wS", [NG, D, GRP * F], fp32, kind="ExternalInput")
    d_AT = nc.dram_tensor("ATS", [NG, D, GRP * F], fp32, kind="ExternalInput")
    d_hnT = nc.dram_tensor("hnTS", [NG, D, GRP * K], fp32, kind="ExternalInput")
    d_hnS = nc.dram_tensor("hnS", [NG, K, GRP * D], fp32, kind="ExternalInput")
    d_hcT = nc.dram_tensor("hcolT", [NG, D, GRP], fp32, kind="ExternalInput")
    d_Wx = nc.dram_tensor("Wxr", [NT, TILE, 3 * D * F], fp32, kind="ExternalInput")
    d_Wn = nc.dram_tensor("Wnr", [NT, TILE, 3 * F * F], fp32, kind="ExternalInput")
    d_h = nc.dram_tensor("hrow", [NT, TILE, D], fp32, kind="ExternalInput")
    d_b = nc.dram_tensor("brow", [NT, TILE, 3 * F], fp32, kind="ExternalInput")
    d_out = nc.dram_tensor("out", [NT, TILE // 2, F], fp32, kind="ExternalOutput")

    with tile.TileContext(nc) as tc:
        with (
            tc.tile_pool(name="const", bufs=1) as cpool,
            tc.tile_pool(name="stat", bufs=2) as spool,     # TensorE stationaries
            tc.tile_pool(name="big", bufs=2) as bpool,      # Wx/Wn gate tiles
            tc.tile_pool(name="vec", bufs=2) as vpool,      # small vectors
            tc.tile_pool(name="ps", bufs=4, space="PSUM") as pspool,
            tc.tile_pool(name="pst", bufs=2, space="PSUM") as psT,
        ):
            ident = cpool.tile([128, 128], fp32)
            make_identity(nc, ident)
            ones32 = cpool.tile([K, 1], fp32)
            nc.vector.memset(ones32, 1.0)
            one1 = cpool.tile([1, 1], fp32)
            nc.vector.memset(one1, 1.0)

            rep_ctx = tc.For_i(0, reps, 1) if reps > 1 else contextlib.nullcontext()
            with rep_ctx:
              for t in range(NT):
                # ---- DVE-side tiles ----
                h_row = vpool.tile([TILE, D], fp32, tag="hrow")
                nc.sync.dma_start(out=h_row, in_=d_h[t])
                b_row = vpool.tile([TILE, 3 * F], fp32, tag="brow")
                nc.sync.dma_start(out=b_row, in_=d_b[t])

                # ---- TensorE stages, per group of 32 pairs ----
                s_cat = vpool.tile([1, TILE], fp32, tag="scat")
                df_row_u = vpool.tile([TILE, F], fp32, tag="dfru")
                for gi in range(GPT):
                    g = t * GPT + gi
                    wS = spool.tile([D, GRP * F], fp32, tag="wS")
                    nc.sync.dma_start(out=wS, in_=d_wS[g])
                    ATS = spool.tile([D, GRP * F], fp32, tag="ATS")
                    nc.sync.dma_start(out=ATS, in_=d_AT[g])
                    hnT = spool.tile([D, GRP * K], fp32, tag="hnT")
                    nc.sync.dma_start(out=hnT, in_=d_hnT[g])
                    hnS = spool.tile([K, GRP * D], fp32, tag="hnS")
                    nc.sync.dma_start(out=hnS, in_=d_hnS[g])
                    hcT = spool.tile([D, GRP], fp32, tag="hcT")
                    nc.sync.dma_start(out=hcT, in_=d_hcT[g])

                    # u = A @ h   -> PSUM [D, GRP]
                    u_ps = pspool.tile([D, GRP], fp32, tag="gps")
                    for i in range(GRP):
                        nc.tensor.matmul(
                            u_ps[:, i : i + 1],
                            ATS[:, i * F : (i + 1) * F],
                            hcT[:, i : i + 1],
                            start=True, stop=True,
                        )
                    u_sb = vpool.tile([D, GRP], fp32, tag="usb")
                    nc.vector.tensor_copy(u_sb, u_ps)

                    # scores = hn @ u -> PSUM [K, GRP]
                    sc_ps = pspool.tile([K, GRP], fp32, tag="gps")
                    for i in range(GRP):
                        nc.tensor.matmul(
                            sc_ps[:, i : i + 1],
                            hnT[:, i * K : (i + 1) * K],
                            u_sb[:, i : i + 1],
                            start=True, stop=True,
                        )
                    # E = exp(scores) (unnormalized)
                    E_sb = vpool.tile([K, GRP], fp32, tag="esb")
                    nc.scalar.activation(
                        out=E_sb, in_=sc_ps,
                        func=mybir.ActivationFunctionType.Exp,
                    )
                    # s = sum_k E -> [1, GRP]
                    s_ps = psT.tile([1, GRP], fp32, tag="tps")
                    nc.tensor.matmul(s_ps, ones32, E_sb, start=True, stop=True)
                    nc.vector.tensor_copy(
                        s_cat[:, gi * GRP : (gi + 1) * GRP], s_ps
                    )

                    # g = hn^T @ E -> [D, GRP]
                    g_ps = pspool.tile([D, GRP], fp32, tag="gps")
                    for i in range(GRP):
                        nc.tensor.matmul(
                            g_ps[:, i : i + 1],
                            hnS[:, i * D : (i + 1) * D],
                            E_sb[:, i : i + 1],
                            start=True, stop=True,
                        )
                    g_sb = vpool.tile([D, GRP], fp32, tag="gsb")
                    nc.vector.tensor_copy(g_sb, g_ps)

                    # df_u = w^T @ g -> [F, GRP]
                    df_ps = pspool.tile([F, GRP], fp32, tag="gps")
                    for i in range(GRP):
                        nc.tensor.matmul(
                            df_ps[:, i : i + 1],
                            wS[:, i * F : (i + 1) * F],
                            g_sb[:, i : i + 1],
                            start=True, stop=True,
                        )
                    df_sb = vpool.tile([F, GRP], fp32, tag="dfsb")
                    nc.vector.tensor_copy(df_sb, df_ps)
                    dft_ps = psT.tile([GRP, F], fp32, tag="tps")
                    nc.tensor.transpose(dft_ps, df_sb, ident[:F, :F])
                    nc.vector.tensor_copy(
                        df_row_u[gi * GRP : (gi + 1) * GRP, :], dft_ps
                    )

                # ---- 1/s as a column [TILE, 1]; df to rows, normalized ----
                rs_cat = vpool.tile([1, TILE], fp32, tag="rscat")
                nc.vector.reciprocal(rs_cat, s_cat)
                rs_ps = psT.tile([TILE, 1], fp32, tag="tps")
                nc.tensor.matmul(rs_ps, rs_cat, one1, start=True, stop=True)
                rs_col = vpool.tile([TILE, 1], fp32, tag="rscol")
                nc.vector.tensor_copy(rs_col, rs_ps)
                df_row = vpool.tile([TILE, F], fp32, tag="dfrow")
                nc.vector.tensor_scalar_mul(df_row, df_row_u, rs_col)

                # ---- GRU gates on DVE (pairs on partitions) ----
                def matvec_row(w_dram_col0, nelem_in, vec_row, tag):
                    """out[p,g] = sum_f vec[p,f] * W[p,f,g]  via TT mul + reduce."""
                    Wt = bpool.tile([TILE, nelem_in * F], fp32, tag="gateW")
                    nc.sync.dma_start(out=Wt, in_=w_dram_col0)
                    prod = bpool.tile([TILE, nelem_in * F], fp32, tag="gateP")
                    nc.vector.tensor_tensor(
                        out=prod.rearrange("p (f g) -> p f g", f=nelem_in),
                        in0=Wt.rearrange("p (f g) -> p f g", f=nelem_in),
                        in1=vec_row.to_broadcast([TILE, nelem_in, F]),
                        op=mybir.AluOpType.mult,
                    )
                    red = vpool.tile([TILE, F], fp32, tag=tag)
                    nc.vector.tensor_reduce(
                        out=red,
                        in_=prod.rearrange("p (f g) -> p g f", f=nelem_in),
                        axis=mybir.AxisListType.X,
                        op=mybir.AluOpType.add,
                    )
                    return red

                Wx_ap = d_Wx[t].rearrange("p (j e) -> p j e", j=3)
                Wn_ap = d_Wn[t].rearrange("p (j e) -> p j e", j=3)
                X0 = matvec_row(Wx_ap[:, 0, :], D, h_row, "X0")
                X1 = matvec_row(Wx_ap[:, 1, :], D, h_row, "X1")
                X2 = matvec_row(Wx_ap[:, 2, :], D, h_row, "X2")
                A0 = matvec_row(Wn_ap[:, 0, :], F, df_row, "A0")
                A1 = matvec_row(Wn_ap[:, 1, :], F, df_row, "A1")

                def gate(x, a, j, func):
                    pre = vpool.tile([TILE, F], fp32, tag=f"pre{j}")
                    nc.vector.tensor_add(pre, x, a)
                    nc.vector.tensor_add(pre, pre, b_row[:, j * F : (j + 1) * F])
                    o = vpool.tile([TILE, F], fp32, tag=f"gate{j}")
                    nc.scalar.activation(out=o, in_=pre, func=func)
                    return o

                Sig = mybir.ActivationFunctionType.Sigmoid
                Rg = gate(X0, A0, 0, Sig)
                Z = gate(X1, A1, 1, Sig)
                rdf = vpool.tile([TILE, F], fp32, tag="rdf")
                nc.vector.tensor_mul(rdf, Rg, df_row)
                A2 = matvec_row(Wn_ap[:, 2, :], F, rdf, "A2")
                Hc = gate(X2, A2, 2, mybir.ActivationFunctionType.Tanh)

                # gru = Hc + Z*(df - Hc)
                gru = vpool.tile([TILE, F], fp32, tag="gru")
                nc.vector.tensor_sub(gru, df_row, Hc)
                nc.vector.tensor_mul(gru, gru, Z)
                nc.vector.tensor_add(gru, gru, Hc)

                # ---- mean over r, tanh, transpose to [nodes, F] ----
                gruT_ps = psT.tile([F, TILE], fp32, tag="tps")
                nc.tensor.transpose(gruT_ps, gru, ident[:TILE, :TILE])
                gruT = vpool.tile([F, TILE], fp32, tag="gruT")
                nc.vector.tensor_copy(gruT, gruT_ps)
                tcol = vpool.tile([F, TILE // 2], fp32, tag="tcol")
                nc.vector.tensor_add(
                    tcol,
                    gruT.rearrange("f (n r) -> f r n", r=2)[:, 0, :],
                    gruT.rearrange("f (n r) -> f r n", r=2)[:, 1, :],
                )
                ocolT = vpool.tile([F, TILE // 2], fp32, tag="ocolT")
                nc.scalar.activation(
                    out=ocolT, in_=tcol,
                    func=mybir.ActivationFunctionType.Tanh, scale=0.5,
                )
                out_ps = psT.tile([TILE // 2, F], fp32, tag="tps")
                nc.tensor.transpose(out_ps, ocolT, ident[:F, :F])
                out_sb = vpool.tile([TILE // 2, F], fp32, tag="outsb")
                nc.vector.tensor_copy(out_sb, out_ps)
                nc.sync.dma_start(out=d_out[t], in_=out_sb)

    nc.compile()
    return nc


def _prep(inputs):
    x = np.asarray(inputs["x"]).astype(np.int64)
    nb = np.asarray(inputs["neighbors"]).astype(np.int64)
    embed = np.asarray(inputs["embed"], dtype=np.float32)
    w = np.asarray(inputs["w"], dtype=np.float32).reshape(P_ALL, D, F)
    qw = np.asarray(inputs["qw"], dtype=np.float32).reshape(P_ALL, F, D2)
    kw = np.asarray(inputs["kw"], dtype=np.float32).reshape(P_ALL, F, D2)
    Wx = np.asarray(inputs["Wx"], dtype=np.float32).reshape(P_ALL, 3 * D * F)
    Wn = np.asarray(inputs["Wn"], dtype=np.float32).reshape(P_ALL, 3 * F * F)
    b = (
        np.asarray(inputs["bx"], dtype=np.float32)
        + np.asarray(inputs["bn"], dtype=np.float32)
    ).reshape(P_ALL, 3 * F)

    h = embed[x]                                   # [N, D]
    hv = h[np.repeat(np.arange(N), R)]             # [P, D]
    hn = h[nb.reshape(P_ALL, K)]                   # [P, K, D]
    A = w @ kw @ qw.transpose(0, 2, 1) @ w.transpose(0, 2, 1)  # [P, D, D]

    in_maps = []
    for c in range(NCORES):
        s = slice(c * PPC, (c + 1) * PPC)
        w_c, A_c, hn_c, hv_c = w[s], A[s], hn[s], hv[s]
        m = {
            # [NG, GRP, D, F] -> [NG, D, GRP*F]
            "wS": np.ascontiguousarray(
                w_c.reshape(NG, GRP, D, F).transpose(0, 2, 1, 3).reshape(NG, D, GRP * F)
            ),
            # A^T: lhsT[i, o] = A[o, i]
            "ATS": np.ascontiguousarray(
                A_c.reshape(NG, GRP, D, D).transpose(0, 3, 1, 2).reshape(NG, D, GRP * D)
            ),
            # hn^T: [d, (p k)]
            "hnTS": np.ascontiguousarray(
                hn_c.reshape(NG, GRP, K, D).transpose(0, 3, 1, 2).reshape(NG, D, GRP * K)
            ),
            # hn: [k, (p d)]
            "hnS": np.ascontiguousarray(
                hn_c.reshape(NG, GRP, K, D).transpose(0, 2, 1, 3).reshape(NG, K, GRP * D)
            ),
            "hcolT": np.ascontiguousarray(
                hv_c.reshape(NG, GRP, D).transpose(0, 2, 1)
            ),
            "Wxr": np.ascontiguousarray(Wx[s].reshape(NT, TILE, 3 * D * F)),
            "Wnr": np.ascontiguousarray(Wn[s].reshape(NT, TILE, 3 * F * F)),
            "hrow": np.ascontiguousarray(hv_c.reshape(NT, TILE, D)),
            "brow": np.ascontiguousarray(b[s].reshape(NT, TILE, 3 * F)),
        }
        in_maps.append(m)
    return in_maps


def kernel(**inputs):
    from concourse.bass_utils import run_bass_kernel_spmd

    if "nc" not in _cache:
        _cache["nc"] = _build()
    in_maps = _prep(inputs)
    res = run_bass_kernel_spmd(_cache["nc"], in_maps, list(range(NCORES)))
    outs = [res.results[c]["out"].reshape(NPC, F) for c in range(NCORES)]
    return np.concatenate(outs, axis=0)



# revision 2
# speedup vs baseline: 1.1488x; 1.1488x over previous
"""Trainium2 Bass kernel for nn_DeepUDI (RGAT+GRU message passing), 8-core SPMD.

Sharding: nodes (dim 0) split across 8 cores; 256 nodes = 512 (node,relation)
pairs per core, processed as 4 tiles of 128 pairs (pairs on SBUF partitions).
The node-state table h=embed[x] is tiny, so the neighbor gather is done on
host while sharding (graph-parallel, no collectives).

Algebraic restructuring (host-side, exact in fp32):
  attention scores_k = hn_k . (A @ h)  with  A = w @ kw @ qw^T @ w^T  [64,64]
  folds qw,kw (16K elems/pair) into A (4K elems/pair).
  df = w^T @ (hn^T @ softmax(scores))

Device design: every per-pair matvec is a DVE broadcast-multiply + reduce
with the pair on the partition axis, so one instruction covers 128 pairs.
All per-pair operands are packed into ONE fp16 mega-tensor (one DMA per
tile, half the HBM traffic of fp32); reductions accumulate in fp32 and the
softmax/GRU arithmetic stays fp32.  ~35 instructions per pair-tile, no
TensorE/PSUM, activations on the Scalar engine.
"""

import numpy as np

N, R, K, D, F, D2 = 2048, 2, 32, 64, 64, 128
NCORES = 8
NPC = N // NCORES        # 256 nodes/core
NTILE = 2                # node-tiles of 128 nodes per core
PT = NTILE * R           # 4 pair-tiles per core
P = 128                  # pairs per tile (partition dim)

# fp16 mega-blob layout (elements per pair)
SZ_A = D * D             # A, (out,in)
SZ_W = D * F             # w^T, (f,d)
SZ_HNK = K * D           # hn, (k,d)
SZ_HND = D * K           # hn^T, (d,k)
SZ_WX = 3 * D * F        # Wx^T, (j,g,d)
SZ_WN = 3 * F * F        # Wn^T, (j,g,f)
SZ_B = 3 * F             # bx+bn, (j,g)
SZ_H = D                 # center h
OFF_A = 0
OFF_W = OFF_A + SZ_A
OFF_HNK = OFF_W + SZ_W
OFF_HND = OFF_HNK + SZ_HNK
OFF_WX = OFF_HND + SZ_HND
OFF_WN = OFF_WX + SZ_WX
OFF_B = OFF_WN + SZ_WN
OFF_H = OFF_B + SZ_B
NW = OFF_H + SZ_H        # 37120

_cache = {}


def _build(reps=1):
    import contextlib
    import concourse.mybir as mybir
    import concourse.tile as tile
    from concourse import bacc

    fp32 = mybir.dt.float32
    fp16 = mybir.dt.float16
    nc = bacc.Bacc(
        "TRN2", target_bir_lowering=False, debug=False, num_devices=NCORES
    )

    d_W = nc.dram_tensor("Wmeg", [PT, P, NW], fp16, kind="ExternalInput")
    d_out = nc.dram_tensor("out", [NTILE, P, F], fp32, kind="ExternalOutput")

    Ax = mybir.AxisListType.X
    Mul = mybir.AluOpType.mult
    Add = mybir.AluOpType.add
    Exp = mybir.ActivationFunctionType.Exp
    Sig = mybir.ActivationFunctionType.Sigmoid
    Tanh = mybir.ActivationFunctionType.Tanh

    with tile.TileContext(nc) as tc:
        with (
            tc.tile_pool(name="wmeg", bufs=2) as wpool,
            tc.tile_pool(name="prod", bufs=1) as ppool,
            tc.tile_pool(name="vec", bufs=2) as vpool,
        ):
            rep_ctx = tc.For_i(0, reps, 1) if reps > 1 else contextlib.nullcontext()
            with rep_ctx:
              for i in range(NTILE):
                grus = []
                for r in range(R):
                    t = NTILE * r + i
                    Wt = wpool.tile([P, NW], fp16, tag="W")
                    nc.sync.dma_start(out=Wt, in_=d_W[t])
                    prod = ppool.tile([P, SZ_WX], fp16, tag="prod")

                    def hB(m):
                        return (
                            Wt[:, OFF_H : OFF_H + D]
                            .unsqueeze(1)
                            .broadcast_to([P, m, D])
                        )

                    def mv(view, n_out, bcast, tag):
                        """out32[p,o] = sum_c view[p,o,c] * bcast[p,o,c]"""
                        nin = view.shape[-1]
                        pr = prod[:, : n_out * nin].rearrange(
                            "p (a b) -> p a b", b=nin
                        )
                        nc.vector.tensor_tensor(out=pr, in0=view, in1=bcast, op=Mul)
                        red = vpool.tile([P, n_out], fp32, tag=tag)
                        nc.vector.tensor_reduce(out=red, in_=pr, axis=Ax, op=Add)
                        return red

                    def f16(x, tag):
                        y = vpool.tile([P, x.shape[-1]], fp16, tag=tag)
                        nc.vector.tensor_copy(y, x)
                        return y

                    # u = A @ h
                    Av = Wt[:, OFF_A : OFF_A + SZ_A].rearrange(
                        "p (o i) -> p o i", o=D
                    )
                    u32 = mv(Av, D, hB(D), "u32")
                    u16 = f16(u32, "u16")

                    # scores_k = hn_k . u ;  E = exp(scores); s = sum_k E
                    hnK = Wt[:, OFF_HNK : OFF_HNK + SZ_HNK].rearrange(
                        "p (k d) -> p k d", k=K
                    )
                    sc32 = mv(hnK, K, u16.unsqueeze(1).broadcast_to([P, K, D]), "sc")
                    E32 = vpool.tile([P, K], fp32, tag="E32")
                    s32 = vpool.tile([P, 1], fp32, tag="s32")
                    nc.scalar.activation(out=E32, in_=sc32, func=Exp, accum_out=s32)
                    rs = vpool.tile([P, 1], fp32, tag="rs")
                    nc.vector.reciprocal(rs, s32)
                    En16 = vpool.tile([P, K], fp16, tag="En16")
                    nc.vector.tensor_scalar_mul(En16, E32, rs)

                    # g = hn^T @ En ;  df = w^T @ g
                    hnD = Wt[:, OFF_HND : OFF_HND + SZ_HND].rearrange(
                        "p (d k) -> p d k", d=D
                    )
                    g32 = mv(hnD, D, En16.unsqueeze(1).broadcast_to([P, D, K]), "g32")
                    g16 = f16(g32, "g16")
                    Wv = Wt[:, OFF_W : OFF_W + SZ_W].rearrange(
                        "p (f d) -> p f d", f=F
                    )
                    df32 = mv(Wv, F, g16.unsqueeze(1).broadcast_to([P, F, D]), "df32")
                    df16 = f16(df32, "df16")

                    # GRU input projections: X = [xr|xz|xh] in one op
                    Wxv = Wt[:, OFF_WX : OFF_WX + SZ_WX].rearrange(
                        "p (a d) -> p a d", d=D
                    )
                    X32 = mv(Wxv, 3 * F, hB(3 * F), "X32")
                    Wn01 = Wt[:, OFF_WN : OFF_WN + 2 * F * F].rearrange(
                        "p (a f) -> p a f", f=F
                    )
                    A01 = mv(
                        Wn01, 2 * F,
                        df16.unsqueeze(1).broadcast_to([P, 2 * F, F]), "A01",
                    )

                    b32 = vpool.tile([P, SZ_B], fp32, tag="b32")
                    nc.vector.tensor_copy(b32, Wt[:, OFF_B : OFF_B + SZ_B])

                    def gate(xoff, a_ap, func, tag):
                        pre = vpool.tile([P, F], fp32, tag=tag + "p")
                        nc.vector.tensor_add(pre, X32[:, xoff : xoff + F], a_ap)
                        nc.vector.tensor_add(pre, pre, b32[:, xoff : xoff + F])
                        o = vpool.tile([P, F], fp32, tag=tag)
                        nc.scalar.activation(out=o, in_=pre, func=func)
                        return o

                    Rg = gate(0, A01[:, 0:F], Sig, "Rg")
                    Z = gate(F, A01[:, F : 2 * F], Sig, "Z")

                    rdf16 = vpool.tile([P, F], fp16, tag="rdf16")
                    nc.vector.tensor_mul(rdf16, Rg, df32)
                    Wn2 = Wt[:, OFF_WN + 2 * F * F : OFF_WN + SZ_WN].rearrange(
                        "p (g f) -> p g f", f=F
                    )
                    A2 = mv(Wn2, F, rdf16.unsqueeze(1).broadcast_to([P, F, F]), "A2")
                    Hc = gate(2 * F, A2, Tanh, "Hc")

                    # gru = Hc + Z*(df - Hc)
                    gru = vpool.tile([P, F], fp32, tag=f"gru{r}")
                    nc.vector.tensor_sub(gru, df32, Hc)
                    nc.vector.tensor_mul(gru, gru, Z)
                    nc.vector.tensor_add(gru, gru, Hc)
                    grus.append(gru)

                # out = tanh(mean over r)
                tsum = vpool.tile([P, F], fp32, tag="tsum")
                nc.vector.tensor_add(tsum, grus[0], grus[1])
                out_sb = vpool.tile([P, F], fp32, tag="out_sb")
                nc.scalar.activation(out=out_sb, in_=tsum, func=Tanh, scale=0.5)
                nc.sync.dma_start(out=d_out[i], in_=out_sb)

    nc.compile()
    return nc


def _prep(inputs):
    x = np.asarray(inputs["x"]).astype(np.int64)
    nbr = np.asarray(inputs["neighbors"]).astype(np.int64)
    embed = np.asarray(inputs["embed"], dtype=np.float32)
    w = np.asarray(inputs["w"], dtype=np.float32)       # [N,R,D,F]
    qw = np.asarray(inputs["qw"], dtype=np.float32)     # [N,R,F,D2]
    kw = np.asarray(inputs["kw"], dtype=np.float32)
    Wx = np.asarray(inputs["Wx"], dtype=np.float32)     # [N,R,3,D,F]
    Wn = np.asarray(inputs["Wn"], dtype=np.float32)
    b = (
        np.asarray(inputs["bx"], dtype=np.float32)
        + np.asarray(inputs["bn"], dtype=np.float32)
    )                                                   # [N,R,3,F]

    h = embed[x]                                        # [N,D]
    hn = h[nbr]                                         # [N,R,K,D]
    wf = w.reshape(N * R, D, F)
    kq = kw.reshape(N * R, F, D2) @ qw.reshape(N * R, F, D2).transpose(0, 2, 1)
    A = (wf @ kq @ wf.transpose(0, 2, 1)).reshape(N, R, D, D)

    def nv(arr):  # [N, X] -> [NCORES, NTILE, P, X]
        return arr.reshape(NCORES, NTILE, P, arr.shape[1])

    blob = np.empty((NCORES, PT, P, NW), np.float16)
    for r in range(R):
        t = NTILE * r
        bl = blob[:, t : t + NTILE]
        bl[..., OFF_A:OFF_W] = nv(A[:, r].reshape(N, SZ_A))
        bl[..., OFF_W:OFF_HNK] = nv(
            w[:, r].transpose(0, 2, 1).reshape(N, SZ_W)
        )
        bl[..., OFF_HNK:OFF_HND] = nv(hn[:, r].reshape(N, SZ_HNK))
        bl[..., OFF_HND:OFF_WX] = nv(
            hn[:, r].transpose(0, 2, 1).reshape(N, SZ_HND)
        )
        bl[..., OFF_WX:OFF_WN] = nv(
            Wx[:, r].transpose(0, 1, 3, 2).reshape(N, SZ_WX)
        )
        bl[..., OFF_WN:OFF_B] = nv(
            Wn[:, r].transpose(0, 1, 3, 2).reshape(N, SZ_WN)
        )
        bl[..., OFF_B:OFF_H] = nv(b[:, r].reshape(N, SZ_B))
        bl[..., OFF_H:NW] = nv(h)
    return [{"Wmeg": blob[c]} for c in range(NCORES)]


def kernel(**inputs):
    from concourse.bass_utils import run_bass_kernel_spmd

    if "nc" not in _cache:
        _cache["nc"] = _build()
    in_maps = _prep(inputs)
    res = run_bass_kernel_spmd(_cache["nc"], in_maps, list(range(NCORES)))
    outs = [res.results[c]["out"].reshape(NPC, F) for c in range(NCORES)]
    return np.concatenate(outs, axis=0)


# revision 10
# speedup vs baseline: 437678.8182x; 380989.2323x over previous
"""Trainium2 Bass kernel for nn_DeepUDI (RGAT+GRU message passing), 8-core SPMD.

Sharding: nodes (dim 0) split across 8 cores; 256 nodes = 512 (node,relation)
pairs per core, processed as 4 tiles of 128 pairs (pairs on SBUF partitions).
The node-state table h=embed[x] is tiny, so the neighbor gather is done on
host while sharding (graph-parallel, no collectives).

Algebraic restructuring (host-side, exact in fp32):
  attention scores_k = hn_k . (A @ h)  with  A = w @ kw @ qw^T @ w^T  [64,64]
  folds qw,kw (16K elems/pair) into A (4K elems/pair).
  df = w^T @ (hn^T @ softmax(scores))

Device design: every per-pair matvec is a DVE broadcast-multiply + reduce
with the pair on the partition axis, so one instruction covers 128 pairs.
All per-pair operands are packed into ONE fp16 mega-tensor (one DMA per
tile, half the HBM traffic of fp32); reductions accumulate in fp32 and the
softmax/GRU arithmetic stays fp32.  ~35 instructions per pair-tile, no
TensorE/PSUM, activations on the Scalar engine.
"""

import numpy as np

N, R, K, D, F, D2 = 2048, 2, 32, 64, 64, 128
NCORES = 8
NPC = N // NCORES        # 256 nodes/core
NTILE = 2                # node-tiles of 128 nodes per core
PT = NTILE * R           # 4 pair-tiles per core
P = 128                  # pairs per tile (partition dim)

# fp16 mega-blob layout (elements per pair)
SZ_A = D * D             # A, (out,in)
SZ_W = D * F             # w^T, (f,d)
SZ_HNK = K * D           # hn, (k,d)
SZ_HND = D * K           # hn^T, (d,k)
SZ_WX = 3 * D * F        # Wx^T, (j,g,d)
SZ_WN = 3 * F * F        # Wn^T, (j,g,f)
SZ_B = 3 * F             # bx+bn, (j,g)
SZ_H = D                 # center h
OFF_A = 0
OFF_W = OFF_A + SZ_A
OFF_HNK = OFF_W + SZ_W
OFF_HND = OFF_HNK + SZ_HNK
OFF_WX = OFF_HND + SZ_HND
OFF_WN = OFF_WX + SZ_WX
OFF_B = OFF_WN + SZ_WN
OFF_H = OFF_B + SZ_B
NW = OFF_H + SZ_H        # 37120

_cache = {}


def _build(reps=1):
    import contextlib
    import concourse.mybir as mybir
    import concourse.tile as tile
    from concourse import bacc

    fp32 = mybir.dt.float32
    fp16 = mybir.dt.float16
    nc = bacc.Bacc(
        "TRN2", target_bir_lowering=False, debug=False, num_devices=NCORES
    )

    d_W = nc.dram_tensor("Wmeg", [PT, P, NW], fp16, kind="ExternalInput")
    d_out = nc.dram_tensor("out", [NTILE, P, F], fp32, kind="ExternalOutput")

    Ax = mybir.AxisListType.X
    Mul = mybir.AluOpType.mult
    Add = mybir.AluOpType.add
    Exp = mybir.ActivationFunctionType.Exp
    Sig = mybir.ActivationFunctionType.Sigmoid
    Tanh = mybir.ActivationFunctionType.Tanh

    Cp = mybir.ActivationFunctionType.Copy

    with tile.TileContext(nc) as tc:
        with (
            tc.tile_pool(name="wmeg", bufs=2) as wpool,
            tc.tile_pool(name="pprod", bufs=2) as gpool,   # Pool-engine prods
            tc.tile_pool(name="ptree", bufs=1) as tpool,   # Pool-engine trees
            tc.tile_pool(name="dprod", bufs=1) as dpool,   # DVE prods/trees
            tc.tile_pool(name="vec", bufs=2) as vpool,
        ):
            rep_ctx = tc.For_i(0, reps, 1) if reps > 1 else contextlib.nullcontext()
            with rep_ctx:
              for i in range(NTILE):
                grus = []
                for r in range(R):
                    t = NTILE * r + i
                    Wt = wpool.tile([P, NW], fp16, tag="W")
                    nc.sync.dma_start(out=Wt, in_=d_W[t])

                    def hB(m):
                        return (
                            Wt[:, OFF_H : OFF_H + D]
                            .unsqueeze(1)
                            .broadcast_to([P, m, D])
                        )

                    def bc(v, m):
                        return v.unsqueeze(1).broadcast_to([P, m, v.shape[-1]])

                    def pool_mult(view, bcast, tag, size=SZ_A):
                        """Pool-engine elementwise product into a fresh fp16 buf."""
                        a, b = view.shape[-2], view.shape[-1]
                        pr = gpool.tile([P, size], fp16, tag=tag)
                        nc.gpsimd.scalar_tensor_tensor(
                            out=pr[:, : a * b].rearrange("p (a b) -> p a b", b=b),
                            in0=view, scalar=1.0, in1=bcast, op0=Mul, op1=Mul,
                        )
                        return pr

                    def dve_mult(view, bcast):
                        """DVE product into the shared big scratch buffer."""
                        a, b = view.shape[-2], view.shape[-1]
                        pr = dpool.tile([P, SZ_A], fp16, tag="pbig")
                        nc.vector.tensor_tensor(
                            out=pr[:, : a * b].rearrange("p (a b) -> p a b", b=b),
                            in0=view, in1=bcast, op=Mul,
                        )
                        return pr

                    def tree(pr, a, c, tag, eng, pool, down_to=4):
                        """Halve contraction c -> down_to with fp16 adds."""
                        cur, cc = pr, c
                        while cc > down_to:
                            nxt = pool.tile([P, a * cc // 2], fp16, tag=f"{tag}{cc}")
                            v = cur[:, : a * cc].rearrange("p (a b) -> p a b", b=cc)
                            o = nxt.rearrange("p (a b) -> p a b", b=cc // 2)
                            if eng == "pool":
                                nc.gpsimd.scalar_tensor_tensor(
                                    out=o, in0=v[:, :, : cc // 2], scalar=1.0,
                                    in1=v[:, :, cc // 2 :], op0=Mul, op1=Add,
                                )
                            else:
                                nc.vector.tensor_tensor(
                                    out=o, in0=v[:, :, : cc // 2],
                                    in1=v[:, :, cc // 2 :], op=Add,
                                )
                            cur, cc = nxt, cc // 2
                        return cur, cc

                    def red32(pr, a, c, tag):
                        """Final fp32 reduce of [P, a, c] -> [P, a] on DVE."""
                        out = vpool.tile([P, a], fp32, tag=tag)
                        nc.vector.tensor_reduce(
                            out=out,
                            in_=pr[:, : a * c].rearrange("p (a b) -> p a b", b=c),
                            axis=Ax, op=Add,
                        )
                        return out

                    def cast16(x, tag):
                        y = vpool.tile([P, x.shape[-1]], fp16, tag=tag)
                        nc.scalar.activation(out=y, in_=x, func=Cp)
                        return y

                    # ---- Pool engine: all h-projections ----
                    Av = Wt[:, OFF_A : OFF_A + SZ_A].rearrange(
                        "p (o i) -> p o i", o=D
                    )
                    pu = pool_mult(Av, hB(D), "pp")          # u products
                    Xt = []
                    for j in range(3):
                        Wxj = Wt[
                            :, OFF_WX + j * D * F : OFF_WX + (j + 1) * D * F
                        ].rearrange("p (g d) -> p g d", g=F)
                        px = pool_mult(Wxj, hB(F), "pp")
                        pxt, cc = tree(px, F, D, "tx", "pool", tpool)
                        Xt.append((pxt, cc))

                    # ---- DVE: attention chain ----
                    u32 = red32(pu, D, D, "u32")             # exact fp32 accum
                    u16 = cast16(u32, "u16")
                    hnK = Wt[:, OFF_HNK : OFF_HNK + SZ_HNK].rearrange(
                        "p (k d) -> p k d", k=K
                    )
                    ps = dve_mult(hnK, bc(u16, K))
                    sc32 = red32(ps, K, D, "sc32")           # exact fp32 accum
                    E32 = vpool.tile([P, K], fp32, tag="E32")
                    s32 = vpool.tile([P, 1], fp32, tag="s32")
                    nc.scalar.activation(out=E32, in_=sc32, func=Exp, accum_out=s32)
                    rs = vpool.tile([P, 1], fp32, tag="rs")
                    nc.vector.reciprocal(rs, s32)
                    En16 = vpool.tile([P, K], fp16, tag="En16")
                    nc.vector.tensor_scalar_mul(En16, E32, rs)

                    hnD = Wt[:, OFF_HND : OFF_HND + SZ_HND].rearrange(
                        "p (d k) -> p d k", d=D
                    )
                    pg = pool_mult(hnD, bc(En16, D), "pg", size=SZ_HND)
                    pgt, cc = tree(pg, D, K, "tg", "dve", dpool)
                    g32 = red32(pgt, D, cc, "g32")
                    g16 = cast16(g32, "g16")

                    Wv = Wt[:, OFF_W : OFF_W + SZ_W].rearrange(
                        "p (f d) -> p f d", f=F
                    )
                    pd = dve_mult(Wv, bc(g16, F))
                    pdt, cc = tree(pd, F, D, "tt", "dve", dpool)
                    df32 = red32(pdt, F, cc, "df32")
                    df16 = cast16(df32, "df16")

                    # ---- GRU ----
                    X32 = [red32(pxt, F, cc, f"X{j}") for j, (pxt, cc) in enumerate(Xt)]

                    def wn_mv(j, vec16, tag):
                        Wnj = Wt[
                            :, OFF_WN + j * F * F : OFF_WN + (j + 1) * F * F
                        ].rearrange("p (g f) -> p g f", g=F)
                        pn = dve_mult(Wnj, bc(vec16, F))
                        pnt, cc = tree(pn, F, F, "tt", "dve", dpool)
                        return red32(pnt, F, cc, tag)

                    A0 = wn_mv(0, df16, "A0")
                    A1 = wn_mv(1, df16, "A1")

                    bv = Wt[:, OFF_B : OFF_B + SZ_B]

                    def gate(j, x_ap, a_ap, func, tag):
                        pre = vpool.tile([P, F], fp32, tag=tag + "p")
                        nc.vector.tensor_add(pre, x_ap, a_ap)
                        nc.vector.tensor_add(pre, pre, bv[:, j * F : (j + 1) * F])
                        o = vpool.tile([P, F], fp32, tag=tag)
                        nc.scalar.activation(out=o, in_=pre, func=func)
                        return o

                    Rg = gate(0, X32[0], A0, Sig, "Rg")
                    Z = gate(1, X32[1], A1, Sig, "Z")

                    rdf16 = vpool.tile([P, F], fp16, tag="rdf16")
                    nc.vector.tensor_mul(rdf16, Rg, df32)
                    A2 = wn_mv(2, rdf16, "A2")
                    Hc = gate(2, X32[2], A2, Tanh, "Hc")

                    # gru = Hc + Z*(df - Hc)
                    gru = vpool.tile([P, F], fp32, tag=f"gru{r}")
                    nc.vector.tensor_sub(gru, df32, Hc)
                    nc.vector.tensor_mul(gru, gru, Z)
                    nc.vector.tensor_add(gru, gru, Hc)
                    grus.append(gru)

                # out = tanh(mean over r)
                tsum = vpool.tile([P, F], fp32, tag="tsum")
                nc.vector.tensor_add(tsum, grus[0], grus[1])
                out_sb = vpool.tile([P, F], fp32, tag="out_sb")
                nc.scalar.activation(out=out_sb, in_=tsum, func=Tanh, scale=0.5)
                nc.sync.dma_start(out=d_out[i], in_=out_sb)

    nc.compile()
    return nc


def _prep(inputs):
    x = np.asarray(inputs["x"]).astype(np.int64)
    nbr = np.asarray(inputs["neighbors"]).astype(np.int64)
    embed = np.asarray(inputs["embed"], dtype=np.float32)
    w = np.asarray(inputs["w"], dtype=np.float32)       # [N,R,D,F]
    qw = np.asarray(inputs["qw"], dtype=np.float32)     # [N,R,F,D2]
    kw = np.asarray(inputs["kw"], dtype=np.float32)
    Wx = np.asarray(inputs["Wx"], dtype=np.float32)     # [N,R,3,D,F]
    Wn = np.asarray(inputs["Wn"], dtype=np.float32)
    b = (
        np.asarray(inputs["bx"], dtype=np.float32)
        + np.asarray(inputs["bn"], dtype=np.float32)
    )                                                   # [N,R,3,F]

    h = embed[x]                                        # [N,D]
    hn = h[nbr]                                         # [N,R,K,D]
    wf = w.reshape(N * R, D, F)
    kq = kw.reshape(N * R, F, D2) @ qw.reshape(N * R, F, D2).transpose(0, 2, 1)
    A = (wf @ kq @ wf.transpose(0, 2, 1)).reshape(N, R, D, D)

    def nv(arr):  # [N, X] -> [NCORES, NTILE, P, X]
        return arr.reshape(NCORES, NTILE, P, arr.shape[1])

    blob = np.empty((NCORES, PT, P, NW), np.float16)
    for r in range(R):
        t = NTILE * r
        bl = blob[:, t : t + NTILE]
        bl[..., OFF_A:OFF_W] = nv(A[:, r].reshape(N, SZ_A))
        bl[..., OFF_W:OFF_HNK] = nv(
            w[:, r].transpose(0, 2, 1).reshape(N, SZ_W)
        )
        bl[..., OFF_HNK:OFF_HND] = nv(hn[:, r].reshape(N, SZ_HNK))
        bl[..., OFF_HND:OFF_WX] = nv(
            hn[:, r].transpose(0, 2, 1).reshape(N, SZ_HND)
        )
        bl[..., OFF_WX:OFF_WN] = nv(
            Wx[:, r].transpose(0, 1, 3, 2).reshape(N, SZ_WX)
        )
        bl[..., OFF_WN:OFF_B] = nv(
            Wn[:, r].transpose(0, 1, 3, 2).reshape(N, SZ_WN)
        )
        bl[..., OFF_B:OFF_H] = nv(b[:, r].reshape(N, SZ_B))
        bl[..., OFF_H:NW] = nv(h)
    return [{"Wmeg": blob[c]} for c in range(NCORES)]


def kernel(**inputs):
    from concourse.bass_utils import run_bass_kernel_spmd

    if "nc" not in _cache:
        _cache["nc"] = _build()
    in_maps = _prep(inputs)
    res = run_bass_kernel_spmd(_cache["nc"], in_maps, list(range(NCORES)))
    outs = [res.results[c]["out"].reshape(NPC, F) for c in range(NCORES)]
    return np.concatenate(outs, axis=0)


# revision 18
# speedup vs baseline: 461696.3559x; 1.0549x over previous
"""Trainium2 Bass kernel for nn_DeepUDI (RGAT+GRU message passing), 8-core SPMD.

Sharding: nodes (dim 0) split across 8 cores; 256 nodes = 512 (node,relation)
pairs per core, processed as 4 tiles of 128 pairs (pairs on SBUF partitions).
The node-state table h=embed[x] is tiny, so the neighbor gather is done on
host while sharding (graph-parallel, no collectives).

Algebraic restructuring (host-side, exact in fp32):
  attention scores_k = hn_k . (A @ h)  with  A = w @ kw @ qw^T @ w^T  [64,64]
  folds qw,kw (16K elems/pair) into A (4K elems/pair).
  df = w^T @ (hn^T @ softmax(scores))

Device design: every per-pair matvec is a DVE broadcast-multiply + reduce
with the pair on the partition axis, so one instruction covers 128 pairs.
All per-pair operands are packed into ONE fp16 mega-tensor (one DMA per
tile, half the HBM traffic of fp32); reductions accumulate in fp32 and the
softmax/GRU arithmetic stays fp32.  ~35 instructions per pair-tile, no
TensorE/PSUM, activations on the Scalar engine.
"""

import numpy as np

N, R, K, D, F, D2 = 2048, 2, 32, 64, 64, 128
NCORES = 8
NPC = N // NCORES        # 256 nodes/core
NTILE = 2                # node-tiles of 128 nodes per core
PT = NTILE * R           # 4 pair-tiles per core
P = 128                  # pairs per tile (partition dim)

# fp16 mega-blob layout (elements per pair): attention fields first so the
# per-tile DMA can be split into an early attention part + later gate part.
SZ_A = D * D             # A, (out,in)
SZ_W = D * F             # w^T, (f,d)
SZ_HNK = K * D           # hn, (k,d)
SZ_HND = D * K           # hn^T, (d,k)
SZ_WX = 3 * D * F        # Wx^T, (j,g,d)
SZ_WN = 3 * F * F        # Wn^T, (j,g,f)
SZ_B = 3 * F             # bx+bn, (j,g)
SZ_H = D                 # center h
OFF_A = 0
OFF_W = OFF_A + SZ_A
OFF_HNK = OFF_W + SZ_W
OFF_HND = OFF_HNK + SZ_HNK
OFF_B = OFF_HND + SZ_HND
OFF_H = OFF_B + SZ_B
NW_ATT = OFF_H + SZ_H    # 12544: attention+bias+h part
OFF_WX = NW_ATT
OFF_WN = OFF_WX + SZ_WX
NW = OFF_WN + SZ_WN      # 37120

_cache = {}


def _build(reps=1):
    import contextlib
    import concourse.mybir as mybir
    import concourse.tile as tile
    from concourse import bacc

    fp32 = mybir.dt.float32
    fp16 = mybir.dt.float16
    nc = bacc.Bacc(
        "TRN2", target_bir_lowering=False, debug=False, num_devices=NCORES
    )

    d_W = nc.dram_tensor("Wmeg", [PT, P, NW], fp16, kind="ExternalInput")
    d_out = nc.dram_tensor("out", [NTILE, P, F], fp32, kind="ExternalOutput")

    Ax = mybir.AxisListType.X
    Mul = mybir.AluOpType.mult
    Add = mybir.AluOpType.add
    Exp = mybir.ActivationFunctionType.Exp
    Sig = mybir.ActivationFunctionType.Sigmoid
    Tanh = mybir.ActivationFunctionType.Tanh

    Cp = mybir.ActivationFunctionType.Copy

    with tile.TileContext(nc) as tc:
        with (
            tc.tile_pool(name="wmeg", bufs=2) as wpool,
            tc.tile_pool(name="pprod", bufs=2) as gpool,   # Pool-engine prods
            tc.tile_pool(name="ptree", bufs=1) as tpool,   # Pool-engine trees
            tc.tile_pool(name="dprod", bufs=1) as dpool,   # DVE prods/trees
            tc.tile_pool(name="vec", bufs=1) as vpool,
        ):
            rep_ctx = tc.For_i(0, reps, 1) if reps > 1 else contextlib.nullcontext()
            with rep_ctx:
              for i in range(NTILE):
                grus = []
                for r in range(R):
                    t = NTILE * r + i
                    Wa = wpool.tile([P, NW_ATT], fp16, tag="Wa")
                    nc.sync.dma_start(out=Wa, in_=d_W[t][:, :NW_ATT])
                    Wg = wpool.tile([P, NW - NW_ATT], fp16, tag="Wg")
                    nc.sync.dma_start(out=Wg, in_=d_W[t][:, NW_ATT:])

                    def hB(m):
                        return (
                            Wa[:, OFF_H : OFF_H + D]
                            .unsqueeze(1)
                            .broadcast_to([P, m, D])
                        )

                    def bc(v, m):
                        return v.unsqueeze(1).broadcast_to([P, m, v.shape[-1]])

                    def pool_mult(view, bcast, tag, size=SZ_A):
                        """Pool-engine elementwise product into a fresh fp16 buf."""
                        a, b = view.shape[-2], view.shape[-1]
                        pr = gpool.tile([P, size], fp16, tag=tag)
                        nc.gpsimd.scalar_tensor_tensor(
                            out=pr[:, : a * b].rearrange("p (a b) -> p a b", b=b),
                            in0=view, scalar=1.0, in1=bcast, op0=Mul, op1=Mul,
                        )
                        return pr

                    def dve_mult(view, bcast):
                        """DVE product into the shared big scratch buffer."""
                        a, b = view.shape[-2], view.shape[-1]
                        pr = dpool.tile([P, SZ_A], fp16, tag="pbig")
                        nc.vector.tensor_tensor(
                            out=pr[:, : a * b].rearrange("p (a b) -> p a b", b=b),
                            in0=view, in1=bcast, op=Mul,
                        )
                        return pr

                    def tree(pr, a, c, tag, eng, pool, down_to=4):
                        """Halve contraction c -> down_to with fp16 adds."""
                        cur, cc = pr, c
                        while cc > down_to:
                            nxt = pool.tile([P, a * cc // 2], fp16, tag=f"{tag}{cc}")
                            v = cur[:, : a * cc].rearrange("p (a b) -> p a b", b=cc)
                            o = nxt.rearrange("p (a b) -> p a b", b=cc // 2)
                            if eng == "pool":
                                nc.gpsimd.scalar_tensor_tensor(
                                    out=o, in0=v[:, :, : cc // 2], scalar=1.0,
                                    in1=v[:, :, cc // 2 :], op0=Mul, op1=Add,
                                )
                            else:
                                nc.vector.tensor_tensor(
                                    out=o, in0=v[:, :, : cc // 2],
                                    in1=v[:, :, cc // 2 :], op=Add,
                                )
                            cur, cc = nxt, cc // 2
                        return cur, cc

                    def red32(pr, a, c, tag):
                        """Final fp32 reduce of [P, a, c] -> [P, a] on DVE."""
                        out = vpool.tile([P, a], fp32, tag=tag)
                        nc.vector.tensor_reduce(
                            out=out,
                            in_=pr[:, : a * c].rearrange("p (a b) -> p a b", b=c),
                            axis=Ax, op=Add,
                        )
                        return out

                    def cast16(x, tag):
                        y = vpool.tile([P, x.shape[-1]], fp16, tag=tag)
                        nc.scalar.activation(out=y, in_=x, func=Cp)
                        return y

                    # ---- Pool engine: all h-projections (u then X0..X2) ----
                    Av = Wa[:, OFF_A : OFF_A + SZ_A].rearrange(
                        "p (o i) -> p o i", o=D
                    )
                    pu = pool_mult(Av, hB(D), "pp")          # u products
                    put, ucc = tree(pu, D, D, "xB", "pool", tpool)
                    Xt = []
                    for j in range(3):
                        Wxj = Wg[
                            :, j * D * F : (j + 1) * D * F
                        ].rearrange("p (g d) -> p g d", g=F)
                        px = pool_mult(Wxj, hB(F), "pp")
                        pxt, cc = tree(px, F, D, "xA" if j % 2 == 0 else "xB",
                                       "pool", tpool)
                        Xt.append((pxt, cc))

                    # ---- DVE: attention chain ----
                    u32 = red32(put, D, ucc, "u32")
                    u16 = cast16(u32, "u16")
                    hnK = Wa[:, OFF_HNK : OFF_HNK + SZ_HNK].rearrange(
                        "p (k d) -> p k d", k=K
                    )
                    ps = dve_mult(hnK, bc(u16, K))
                    sc32 = red32(ps, K, D, "sc32")           # exact fp32 accum
                    E32 = vpool.tile([P, K], fp32, tag="E32")
                    s32 = vpool.tile([P, 1], fp32, tag="s32")
                    nc.scalar.activation(out=E32, in_=sc32, func=Exp, accum_out=s32)
                    rs = vpool.tile([P, 1], fp32, tag="rs")
                    nc.vector.reciprocal(rs, s32)
                    En16 = vpool.tile([P, K], fp16, tag="En16")
                    nc.vector.tensor_scalar_mul(En16, E32, rs)
                    X32 = [None, None, None]
                    X32[0] = red32(Xt[0][0], F, Xt[0][1], "X0")

                    hnD = Wa[:, OFF_HND : OFF_HND + SZ_HND].rearrange(
                        "p (d k) -> p d k", d=D
                    )
                    pg = pool_mult(hnD, bc(En16, D), "pp")
                    pgt, cc = tree(pg, D, K, "tg", "dve", dpool)
                    g32 = red32(pgt, D, cc, "g32")
                    g16 = cast16(g32, "g16")
                    X32[1] = red32(Xt[1][0], F, Xt[1][1], "X1")

                    Wv = Wa[:, OFF_W : OFF_W + SZ_W].rearrange(
                        "p (f d) -> p f d", f=F
                    )
                    pd = dve_mult(Wv, bc(g16, F))
                    pdt, cc = tree(pd, F, D, "tt", "dve", dpool)
                    df32 = red32(pdt, F, cc, "df32")
                    df16 = cast16(df32, "df16")
                    X32[2] = red32(Xt[2][0], F, Xt[2][1], "X2")

                    # ---- GRU ----
                    def wn_mv(j, vec16, tag):
                        Wnj = Wg[
                            :, SZ_WX + j * F * F : SZ_WX + (j + 1) * F * F
                        ].rearrange("p (g f) -> p g f", g=F)
                        pn = dve_mult(Wnj, bc(vec16, F))
                        pnt, cc = tree(pn, F, F, "tt", "dve", dpool)
                        return red32(pnt, F, cc, tag)

                    A0 = wn_mv(0, df16, "A0")
                    A1 = wn_mv(1, df16, "A1")

                    bv = Wa[:, OFF_B : OFF_B + SZ_B]

                    def gate(j, x_ap, a_ap, func, tag):
                        pre = vpool.tile([P, F], fp32, tag=tag + "p")
                        nc.vector.tensor_add(pre, x_ap, a_ap)
                        nc.vector.tensor_add(pre, pre, bv[:, j * F : (j + 1) * F])
                        o = vpool.tile([P, F], fp32, tag=tag)
                        nc.scalar.activation(out=o, in_=pre, func=func)
                        return o

                    Rg = gate(0, X32[0], A0, Sig, "Rg")
                    Z = gate(1, X32[1], A1, Sig, "Z")

                    rdf16 = vpool.tile([P, F], fp16, tag="rdf16")
                    nc.vector.tensor_mul(rdf16, Rg, df32)
                    A2 = wn_mv(2, rdf16, "A2")
                    Hc = gate(2, X32[2], A2, Tanh, "Hc")

                    # gru = Hc + Z*(df - Hc)
                    gru = vpool.tile([P, F], fp32, tag=f"gru{r}")
                    nc.vector.tensor_sub(gru, df32, Hc)
                    nc.vector.tensor_mul(gru, gru, Z)
                    nc.vector.tensor_add(gru, gru, Hc)
                    grus.append(gru)

                # out = tanh(mean over r)
                tsum = vpool.tile([P, F], fp32, tag="tsum")
                nc.vector.tensor_add(tsum, grus[0], grus[1])
                out_sb = vpool.tile([P, F], fp32, tag="out_sb")
                nc.scalar.activation(out=out_sb, in_=tsum, func=Tanh, scale=0.5)
                nc.sync.dma_start(out=d_out[i], in_=out_sb)

    nc.compile()
    return nc


def _prep(inputs):
    x = np.asarray(inputs["x"]).astype(np.int64)
    nbr = np.asarray(inputs["neighbors"]).astype(np.int64)
    embed = np.asarray(inputs["embed"], dtype=np.float32)
    w = np.asarray(inputs["w"], dtype=np.float32)       # [N,R,D,F]
    qw = np.asarray(inputs["qw"], dtype=np.float32)     # [N,R,F,D2]
    kw = np.asarray(inputs["kw"], dtype=np.float32)
    Wx = np.asarray(inputs["Wx"], dtype=np.float32)     # [N,R,3,D,F]
    Wn = np.asarray(inputs["Wn"], dtype=np.float32)
    b = (
        np.asarray(inputs["bx"], dtype=np.float32)
        + np.asarray(inputs["bn"], dtype=np.float32)
    )                                                   # [N,R,3,F]

    h = embed[x]                                        # [N,D]
    hn = h[nbr]                                         # [N,R,K,D]
    wf = w.reshape(N * R, D, F)
    kq = kw.reshape(N * R, F, D2) @ qw.reshape(N * R, F, D2).transpose(0, 2, 1)
    A = (wf @ kq @ wf.transpose(0, 2, 1)).reshape(N, R, D, D)

    def nv(arr):  # [N, X] -> [NCORES, NTILE, P, X]
        return arr.reshape(NCORES, NTILE, P, arr.shape[1])

    blob = np.empty((NCORES, PT, P, NW), np.float16)
    for r in range(R):
        t = NTILE * r
        bl = blob[:, t : t + NTILE]
        bl[..., OFF_A : OFF_A + SZ_A] = nv(A[:, r].reshape(N, SZ_A))
        bl[..., OFF_W : OFF_W + SZ_W] = nv(
            w[:, r].transpose(0, 2, 1).reshape(N, SZ_W)
        )
        bl[..., OFF_HNK : OFF_HNK + SZ_HNK] = nv(hn[:, r].reshape(N, SZ_HNK))
        bl[..., OFF_HND : OFF_HND + SZ_HND] = nv(
            hn[:, r].transpose(0, 2, 1).reshape(N, SZ_HND)
        )
        bl[..., OFF_B : OFF_B + SZ_B] = nv(b[:, r].reshape(N, SZ_B))
        bl[..., OFF_H : OFF_H + SZ_H] = nv(h)
        bl[..., OFF_WX : OFF_WX + SZ_WX] = nv(
            Wx[:, r].transpose(0, 1, 3, 2).reshape(N, SZ_WX)
        )
        bl[..., OFF_WN : OFF_WN + SZ_WN] = nv(
            Wn[:, r].transpose(0, 1, 3, 2).reshape(N, SZ_WN)
        )
    return [{"Wmeg": blob[c]} for c in range(NCORES)]


def kernel(**inputs):
    from concourse.bass_utils import run_bass_kernel_spmd

    if "nc" not in _cache:
        _cache["nc"] = _build()
    in_maps = _prep(inputs)
    res = run_bass_kernel_spmd(_cache["nc"], in_maps, list(range(NCORES)))
    outs = [res.results[c]["out"].reshape(NPC, F) for c in range(NCORES)]
    return np.concatenate(outs, axis=0)
